# revision 12
# baseline (speedup 1.0000x reference)
"""Trainium2 Bass kernel for nn_CrossCategoryLoss.

loss(row) = sum_t relu(log_a[A_t] + log_b[B_t] - c_t)
  with c_t = log_g[G_t] (pos) or log(1 - exp(log_g[G_t])) (not).

Math (exact up to fp16 rounding; validated vs fp64: norm-rel ~1.5e-3):
  term = relu((alpha[A] + beta[B]) - q'), q' = q - S,
  S = lsg - lsa - lsb;  q = gamma[G] (pos) | ln(sum_g - e^gamma[G]) (not).
  Exps carry a bias of -ln64 so fp16 holds the sums and the Sa*Sb
  product; the scaling cancels exactly via the ACT Ln `scale` args
  (lsg: ln(64*Sg'), lsab: ln(4096*Sa'Sb'), wl: ln(64*wpre')), so the
  pair sums and q's need no compensation constants.

Engine split (per tile, [128 part, R rows], R=256, double-buffered):
  ACT : 3x exp (fp32 in -> fp16 transposed [P,8,R] out), lsg, lsab,
        batched wl, relu of d36[0:10].
  DVE : softmax sums as batched in-place tree adds over the 8-axis
        (2x fp16; reduce_sum would always run 1x), sab, S, wpre, qnot,
        d36 in 5 batched 2x subs, relu of d36[10:36] as one 4x
        tensor_scalar, fp16 pairwise sum tree.
  Pool: 14 pair sums (7 batched TT adds, fp32 in fp16 out) and qpos
        (TT sub) -- the ops walrus codegen accepts on Pool.

Sharding: pure data-parallel over 8 cores; each core handles B/8 rows.
"""

import numpy as np

import concourse.bass as bass
import concourse.bacc as bacc
import concourse.mybir as mybir
from concourse.tile import TileContext
from concourse.bass_utils import run_bass_kernel_spmd

N_CORES = 8
B = 4194304
B_CORE = B // N_CORES          # 524288 rows per core
P = 128                        # partitions
ROWS_PER_PART = B_CORE // P    # 4096
R = 256                        # rows per partition per tile
N_TILES = ROWS_PER_PART // R   # 16

F32 = mybir.dt.float32
F16 = mybir.dt.float16
AF = mybir.ActivationFunctionType
OP = mybir.AluOpType

LN64 = float(np.log(64.0))

# Pair slots, ordered so every q-group's pair set is a contiguous slot
# range (one batched subtract per q-group):
#   slots: 0 (0,4) 1 (0,6) 2 (2,4) 3 (4,0) 4 (4,2)   X1
#          5 (1,5) 6 (1,6) 7 (2,5) 8 (5,1) 9 (5,2)   X2
#          10 (2,7) 11 (7,2) 12 (2,6) 13 (6,2)       tail
# Batched pair instructions: (slot0, slotstride, n, a0, astride, b0, bstride)
_PAIR_BATCHES = [
    (0, 1, 2, 0, 0, 4, 2),     # (0,4),(0,6)
    (2, 1, 2, 2, 2, 4, -4),    # (2,4),(4,0)
    (4, 5, 2, 4, 1, 2, 0),     # (4,2),(5,2)  [slots 4 and 9]
    (5, 1, 2, 1, 0, 5, 1),     # (1,5),(1,6)
    (7, 1, 2, 2, 3, 5, -4),    # (2,5),(5,1)
    (10, 1, 2, 2, 5, 7, -5),   # (2,7),(7,2)
    (12, 1, 2, 2, 4, 6, -4),   # (2,6),(6,2)
]
# q7 tile slabs: 0..3 = qpos g4..g7, 4..6 = qnot w0..w2.
# d36 slots: [0:5]=g4, [5:10]=w1, [10:22]=w2, [22:27]=g5, [27:32]=w0,
#            [32:34]=g7, [34:36]=g6  (tree pairs slab i with i+18).


def _ap(base, offset, dims):
    """Build an AP from a tile's base AP: partition dim + given free dims."""
    a = base
    return bass.AP(tensor=a.tensor, offset=a.offset + offset, ap=[a.ap[0]] + dims)


def _bcast_mid(ap, n):
    """[P, R] access pattern -> [P, n, R] with a zero-stride middle dim."""
    a = ap[:, :]
    return bass.AP(tensor=a.tensor, offset=a.offset,
                   ap=[a.ap[0], [0, n], a.ap[1]])


def build_kernel(reps: int = 1) -> bass.Bass:
    nc = bacc.Bacc("TRN2", target_bir_lowering=False, debug=False,
                   num_devices=N_CORES)

    # Restrict the ACT table chooser to one set holding Exp, Ln and Relu,
    # so no LoadActFuncSet reloads (~1.3us each) occur mid-kernel.
    _orig_tables = bacc.get_activation_tables

    def _one_set(arch):
        return {
            name: (fns if name == "natural_log_exp_and_others" else set())
            for name, fns in _orig_tables(arch).items()
        }

    bacc.get_activation_tables = _one_set
    try:
        return _build_body(nc, reps)
    finally:
        bacc.get_activation_tables = _orig_tables


def _build_body(nc, reps: int) -> bass.Bass:

    a_d = nc.dram_tensor("alpha_logits", [B_CORE, 8], F32, kind="ExternalInput")
    b_d = nc.dram_tensor("beta_logits", [B_CORE, 8], F32, kind="ExternalInput")
    g_d = nc.dram_tensor("gamma_logits", [B_CORE, 8], F32, kind="ExternalInput")
    o_d = nc.dram_tensor("loss", [B_CORE], F32, kind="ExternalOutput")

    a_v = a_d[:].rearrange("(p n) k -> p n k", p=P)
    b_v = b_d[:].rearrange("(p n) k -> p n k", p=P)
    g_v = g_d[:].rearrange("(p n) k -> p n k", p=P)
    o_v = o_d[:].rearrange("(p n) -> p n", p=P)

    with TileContext(nc) as tc:
        import contextlib
        rep_loop = tc.For_i(0, reps, 1) if reps > 1 else contextlib.nullcontext()
        with (
            rep_loop,
            tc.tile_pool(name="io", bufs=2) as io,
            tc.tile_pool(name="epool", bufs=2) as epool,
            tc.tile_pool(name="work", bufs=2) as work,
            tc.tile_pool(name="outp", bufs=2) as outp,
            tc.tile_pool(name="constp", bufs=1) as constp,
        ):
            nln64 = constp.tile([P, 1], F32, tag="nln64")
            nc.gpsimd.memset(nln64, -LN64)
            for j in range(N_TILES):
                sl = slice(j * R, (j + 1) * R)

                a_t = io.tile([P, R, 8], F32, tag="a")
                b_t = io.tile([P, R, 8], F32, tag="b")
                g_t = io.tile([P, R, 8], F32, tag="g")
                nc.sync.dma_start(out=a_t, in_=a_v[:, sl, :])
                nc.sync.dma_start(out=b_t, in_=b_v[:, sl, :])
                nc.sync.dma_start(out=g_t, in_=g_v[:, sl, :])

                # --- stage 1: exps, transposed into e24 [P, 24, R] fp16 ---
                # slab layout: 0-7 e^a', 8-15 e^b', 16-23 e^g'  (x' = x-ln64)
                # gamma first: it gates the longest chain (wpre/wl/qnot).
                e24 = epool.tile([P, 24, R], F16, tag="e24")
                e_base = e24[:, :, :]
                for t, x_t in ((2, g_t), (0, a_t), (1, b_t)):
                    # out[p, r, k] = e24[p, 8t+k, r]: free dims (R, 8)
                    ov = _ap(e_base, t * 8 * R, [[1, R], [R, 8]])
                    nc.scalar.activation(out=ov, in_=x_t, func=AF.Exp,
                                         bias=nln64[:, :])

                # --- stage 2: softmax sums via in-place tree over the
                # 8-axis (high-half accumulate keeps e^g'[0:3] intact).
                # gamma tree first (unblocks wpre/lsg), then a|b tree.
                g_hi4 = _ap(e_base, 20 * R, [[R, 4], [1, R]])
                g_lo4 = _ap(e_base, 16 * R, [[R, 4], [1, R]])
                nc.vector.tensor_add(g_hi4, g_hi4, g_lo4)
                g_hi2 = _ap(e_base, 22 * R, [[R, 2], [1, R]])
                g_lo2 = _ap(e_base, 20 * R, [[R, 2], [1, R]])
                nc.vector.tensor_add(g_hi2, g_hi2, g_lo2)
                g_hi1 = _ap(e_base, 23 * R, [[1, R]])
                g_lo1 = _ap(e_base, 22 * R, [[1, R]])
                nc.vector.tensor_add(g_hi1, g_hi1, g_lo1)
                sg = _ap(e_base, 23 * R, [[1, R]])

                ab_hi4 = _ap(e_base, 4 * R, [[8 * R, 2], [R, 4], [1, R]])
                ab_lo4 = _ap(e_base, 0, [[8 * R, 2], [R, 4], [1, R]])
                nc.vector.tensor_add(ab_hi4, ab_hi4, ab_lo4)
                ab_hi2 = _ap(e_base, 6 * R, [[8 * R, 2], [R, 2], [1, R]])
                ab_lo2 = _ap(e_base, 4 * R, [[8 * R, 2], [R, 2], [1, R]])
                nc.vector.tensor_add(ab_hi2, ab_hi2, ab_lo2)
                ab_hi1 = _ap(e_base, 7 * R, [[8 * R, 2], [1, R]])
                ab_lo1 = _ap(e_base, 6 * R, [[8 * R, 2], [1, R]])
                nc.vector.tensor_add(ab_hi1, ab_hi1, ab_lo1)
                sa = _ap(e_base, 7 * R, [[1, R]])    # Sa' = Sa/64
                sb = _ap(e_base, 15 * R, [[1, R]])

                # --- stage 3: S and the q7 tile ---
                # wpre/wl early (gamma-only inputs): longest chain.
                wpre = work.tile([P, 3, R], F16, tag="wpre")
                eg_lo = _ap(e_base, 16 * R, [[R, 3], [1, R]])
                nc.vector.tensor_sub(wpre, _bcast_mid(sg, 3), eg_lo)
                wl = work.tile([P, 3, R], F16, tag="wl")
                nc.scalar.activation(out=wl, in_=wpre, func=AF.Ln, scale=64.0)
                lsg = work.tile([P, R], F16, tag="lsg")
                nc.scalar.activation(out=lsg, in_=sg, func=AF.Ln, scale=64.0)

                sab = work.tile([P, R], F16, tag="sab")
                nc.vector.tensor_mul(sab, sa, sb)            # (SaSb)/4096
                lsab = work.tile([P, R], F16, tag="lsab")
                nc.scalar.activation(out=lsab, in_=sab, func=AF.Ln,
                                     scale=4096.0)
                s_t = work.tile([P, R], F16, tag="S")        # true S
                nc.vector.tensor_sub(s_t, lsg, lsab)

                # q7: slabs 0..3 qpos (Pool TT), 4..6 qnot (DVE TT)
                q7 = work.tile([P, 7, R], F16, tag="q7")
                g_hi = g_t[:, :, :]
                g_pos = bass.AP(tensor=g_hi.tensor, offset=g_hi.offset + 4,
                                ap=[g_hi.ap[0], [1, 4], [8, R]])
                nc.gpsimd.tensor_tensor(out=q7[:, 0:4, :], in0=g_pos,
                                        in1=_bcast_mid(s_t, 4),
                                        op=OP.subtract)
                nc.vector.tensor_sub(q7[:, 4:7, :], wl, _bcast_mid(s_t, 3))

                # --- stage 4: pair sums on Pool (fp32 in, fp16 out),
                # p = a[A] + b[B], batched TT adds over slot runs.
                p14 = work.tile([P, 14, R], F16, tag="p14")
                a_b, b_b = a_t[:, :, :], b_t[:, :, :]
                p_b = p14[:, :, :]
                for (s0, sstr, n, a0, astr, b0, bstr) in _PAIR_BATCHES:
                    av = bass.AP(tensor=a_b.tensor, offset=a_b.offset + a0,
                                 ap=[a_b.ap[0], [astr, n], [8, R]])
                    bv = bass.AP(tensor=b_b.tensor, offset=b_b.offset + b0,
                                 ap=[b_b.ap[0], [bstr, n], [8, R]])
                    ov = _ap(p_b, s0 * R, [[sstr * R, n], [1, R]])
                    nc.gpsimd.tensor_tensor(out=ov, in0=av, in1=bv, op=OP.add)

                # --- stage 5: d36, relu, tree sum ---
                # High half (slabs 10:36) first; ACT relus [0:10], DVE
                # ts-relus [10:36] at 4x.
                d36 = work.tile([P, 36, R], F16, tag="d36")
                q_b = q7[:, :, :]

                # w2: D[10:22] = P[0:12] - q[6]
                nc.vector.tensor_sub(d36[:, 10:22, :], p14[:, 0:12, :],
                                     _bcast_mid(q7[:, 6, :], 12))
                # g5: D[22:27] = P[5:10] - q[1]
                nc.vector.tensor_sub(d36[:, 22:27, :], p14[:, 5:10, :],
                                     _bcast_mid(q7[:, 1, :], 5))
                # ACT relus [10:27] as soon as written (ACT has headroom)
                nc.scalar.activation(out=d36[:, 10:27, :],
                                     in_=d36[:, 10:27, :], func=AF.Relu)
                # w0: D[27:32] = P[5:10] - q[4]
                nc.vector.tensor_sub(d36[:, 27:32, :], p14[:, 5:10, :],
                                     _bcast_mid(q7[:, 4, :], 5))
                # g7,g6: D[32:36] = P[10:14] - q{3,3,2,2}
                q76 = bass.AP(tensor=q_b.tensor, offset=q_b.offset + 3 * R,
                              ap=[q_b.ap[0], [-R, 2], [0, 2], [1, R]])
                nc.vector.tensor_sub(d36[:, 32:36, :], p14[:, 10:14, :], q76)
                # relu [27:36]: one 4x tensor_scalar on DVE
                nc.vector.tensor_scalar(
                    out=d36[:, 27:36, :], in0=d36[:, 27:36, :],
                    scalar1=0.0, scalar2=None, op0=OP.max)
                # g4,w1: D[0:10] = P[0:5] x2 - q{0 (g4), 5 (w1)}
                q04 = bass.AP(tensor=q_b.tensor, offset=q_b.offset,
                              ap=[q_b.ap[0], [5 * R, 2], [0, 5], [1, R]])
                p05 = bass.AP(tensor=p_b.tensor, offset=p_b.offset,
                              ap=[p_b.ap[0], [0, 2], [R, 5], [1, R]])
                nc.vector.tensor_sub(
                    _ap(d36[:, :, :], 0, [[5 * R, 2], [R, 5], [1, R]]),
                    p05, q04)
                nc.scalar.activation(out=d36[:, 0:10, :],
                                     in_=d36[:, 0:10, :], func=AF.Relu)

                # fp16 pairwise tree: 36 -> 18 -> 9 (Pool) -> (4+1) -> 2 -> 1
                nc.vector.tensor_add(d36[:, 0:18, :], d36[:, 0:18, :],
                                     d36[:, 18:36, :])
                nc.gpsimd.tensor_tensor(out=d36[:, 0:9, :], in0=d36[:, 0:9, :],
                                        in1=d36[:, 9:18, :], op=OP.add)
                nc.vector.tensor_add(d36[:, 0:4, :], d36[:, 0:4, :],
                                     d36[:, 4:8, :])
                nc.vector.tensor_add(d36[:, 0:2, :], d36[:, 0:2, :],
                                     d36[:, 2:4, :])
                nc.vector.tensor_add(d36[:, 0, :], d36[:, 0, :], d36[:, 1, :])
                loss_t = outp.tile([P, R], F32, tag="loss")
                nc.vector.tensor_add(loss_t, d36[:, 0, :], d36[:, 8, :])
                nc.sync.dma_start(out=o_v[:, sl], in_=loss_t)

    nc.compile()
    return nc


_NC_CACHE = None


def _get_nc():
    global _NC_CACHE
    if _NC_CACHE is None:
        _NC_CACHE = build_kernel()
    return _NC_CACHE


def kernel(alpha_logits, beta_logits, gamma_logits, _trace=False):
    nc = _get_nc()
    in_maps = []
    for c in range(N_CORES):
        sl = slice(c * B_CORE, (c + 1) * B_CORE)
        in_maps.append({
            "alpha_logits": np.ascontiguousarray(alpha_logits[sl]),
            "beta_logits": np.ascontiguousarray(beta_logits[sl]),
            "gamma_logits": np.ascontiguousarray(gamma_logits[sl]),
        })
    res = run_bass_kernel_spmd(nc, in_maps, core_ids=list(range(N_CORES)),
                               trace=_trace)
    out = np.concatenate([r["loss"] for r in res.results])
    if _trace:
        kernel.last_result = res
    return out


# revision 13
# speedup vs baseline: 1.3108x; 1.3108x over previous
"""Trainium2 Bass kernel for nn_CrossCategoryLoss.

loss(row) = sum_t relu(log_a[A_t] + log_b[B_t] - c_t)
  with c_t = log_g[G_t] (pos) or log(1 - exp(log_g[G_t])) (not).

Rewrites used (all per-row, exact in fp32 up to rounding):
  log_a[i] = alpha[i] - lsa,  lsa = ln(sum_j exp(alpha[j]))  (no max-sub
  needed: inputs are N(0,1), |x| < ~6, exp is safe in fp32)
  log(1-exp(log_g[k])) = ln(sum_g - exp(gamma[k])) - lsg
  term_t = relu(alpha[A] + beta[B] - q_t + S),  S = lsg - lsa - lsb
         = relu(p_AB - q'_t)   with p_AB = alpha[A]+beta[B], q'_t = q_t - S
    q_t = gamma[G]  (pos)  or  ln(sum_g - exp(gamma[G]))  (not)

Sharding: pure data-parallel over 8 cores; each core handles B/8 rows.
Per-core layout: rows viewed as [128 partitions, 4096 rows], tiles of
R rows per partition.
"""

import numpy as np

import concourse.bass as bass
import concourse.bacc as bacc
import concourse.mybir as mybir
from concourse.tile import TileContext
from concourse.bass_utils import run_bass_kernel_spmd

N_CORES = 8
B = 4194304
B_CORE = B // N_CORES          # 524288 rows per core
P = 128                        # partitions
ROWS_PER_PART = B_CORE // P    # 4096
R = 512                        # rows per partition per tile
N_TILES = ROWS_PER_PART // R   # 8

F32 = mybir.dt.float32
F16 = mybir.dt.float16
LN64 = float(np.log(64.0))
AX = mybir.AxisListType
AF = mybir.ActivationFunctionType
OP = mybir.AluOpType

# (alpha_idx, beta_idx, gamma_idx, is_not) - 36 constraint terms.
_TERMS = [
    (0, 4, 4, 0), (0, 4, 1, 1), (0, 4, 2, 1),
    (0, 6, 4, 0), (0, 6, 1, 1), (0, 6, 2, 1),
    (1, 5, 5, 0), (1, 5, 0, 1), (1, 5, 2, 1),
    (1, 6, 5, 0), (1, 6, 0, 1), (1, 6, 2, 1),
    (2, 4, 4, 0), (2, 4, 1, 1), (2, 4, 2, 1),
    (2, 5, 5, 0), (2, 5, 0, 1), (2, 5, 2, 1),
    (2, 6, 6, 0), (2, 7, 7, 0), (2, 7, 2, 1),
    (4, 0, 4, 0), (4, 0, 1, 1), (4, 0, 2, 1),
    (4, 2, 4, 0), (4, 2, 1, 1), (4, 2, 2, 1),
    (5, 1, 5, 0), (5, 1, 0, 1), (5, 1, 2, 1),
    (5, 2, 5, 0), (5, 2, 0, 1), (5, 2, 2, 1),
    (6, 2, 6, 0), (7, 2, 7, 0), (7, 2, 2, 1),
]

# Group terms by (a, b) pair, preserving first-appearance order.
_PAIRS: list[tuple[int, int]] = []
_PAIR_TERMS: dict[tuple[int, int], list[tuple[int, int]]] = {}
for _a, _b, _g, _n in _TERMS:
    if (_a, _b) not in _PAIR_TERMS:
        _PAIRS.append((_a, _b))
        _PAIR_TERMS[(_a, _b)] = []
    _PAIR_TERMS[(_a, _b)].append((_g, _n))

_NOT_GS = sorted({g for _, _, g, n in _TERMS if n})      # [0, 1, 2]
_POS_GS = sorted({g for _, _, g, n in _TERMS if not n})  # [4, 5, 6, 7]

# fp16 for the term stage: 16-bit dtype unlocks the DVE 2x_1P perf mode on
# tensor_tensor / tensor_scalar ops (fp32 is capped at 1x). Simulated error
# vs fp64 reference: norm-rel ~5e-4, absmax/scale ~6e-4. Stage 1 (exp sums,
# logs, S) stays fp32.
TERM_DT = mybir.dt.float16

# Pair slots in the P tile, ordered so every q-group's pair set is a
# contiguous slot range (enables one batched subtract per q-group):
_PAIR_SLOTS = [
    (0, 4), (0, 6), (2, 4), (4, 0), (4, 2),      # X1: q-triple {g4, w1, w2}
    (1, 5), (1, 6), (2, 5), (5, 1), (5, 2),      # X2: q-triple {g5, w0, w2}
    (2, 7), (7, 2), (2, 6), (6, 2),              # tail
]
# (q_key, pair_slot_range, d_slot_start): D[d0:d0+n] = P[p0:p1] - q
_QGROUPS = [
    ((4, 0), 0, 5, 0),     # g4:  D[0:5]   = P[0:5]  - q_g4
    ((1, 1), 0, 5, 5),     # w1:  D[5:10]  = P[0:5]  - q_w1
    ((2, 1), 0, 12, 10),   # w2:  D[10:22] = P[0:12] - q_w2
    ((5, 0), 5, 10, 22),   # g5:  D[22:27] = P[5:10] - q_g5
    ((0, 1), 5, 10, 27),   # w0:  D[27:32] = P[5:10] - q_w0
    ((7, 0), 10, 12, 32),  # g7:  D[32:34] = P[10:12]- q_g7
    ((6, 0), 12, 14, 34),  # g6:  D[34:36] = P[12:14]- q_g6
]


def _bcast_mid(ap, n):
    """[P, R] access pattern -> [P, n, R] with a zero-stride middle dim."""
    a = ap[:, :]
    return bass.AP(tensor=a.tensor, offset=a.offset,
                   ap=[a.ap[0], [0, n], a.ap[1]])


def _swap_free(ap):
    """View a [P, K, R] tile iterated as [P, R, K] (same memory)."""
    a = ap[:, :, :]
    return bass.AP(tensor=a.tensor, offset=a.offset,
                   ap=[a.ap[0], a.ap[2], a.ap[1]])


def build_kernel(reps: int = 1) -> bass.Bass:
    nc = bacc.Bacc("TRN2", target_bir_lowering=False, debug=False,
                   num_devices=N_CORES)

    # The default table chooser alternates ACT function-table sets between
    # Exp and Ln ops (measured: 18 LoadActFuncSet = ~45us of ~2.7us reloads
    # per kernel). All functions used here (Exp, Ln, Relu) coexist in
    # "natural_log_exp_and_others", so restrict the chooser to that set.
    # Other entries are blanked (not removed) to keep act_func_set_id
    # indices aligned with act_info.json.
    _orig_tables = bacc.get_activation_tables

    def _one_set(arch):
        return {
            name: (fns if name == "natural_log_exp_and_others" else set())
            for name, fns in _orig_tables(arch).items()
        }

    bacc.get_activation_tables = _one_set
    try:
        return _build_body(nc, reps)
    finally:
        bacc.get_activation_tables = _orig_tables


def _build_body(nc, reps: int) -> bass.Bass:

    a_d = nc.dram_tensor("alpha_logits", [B_CORE, 8], F32, kind="ExternalInput")
    b_d = nc.dram_tensor("beta_logits", [B_CORE, 8], F32, kind="ExternalInput")
    g_d = nc.dram_tensor("gamma_logits", [B_CORE, 8], F32, kind="ExternalInput")
    o_d = nc.dram_tensor("loss", [B_CORE], F32, kind="ExternalOutput")

    a_v = a_d[:].rearrange("(p n) k -> p n k", p=P)
    b_v = b_d[:].rearrange("(p n) k -> p n k", p=P)
    g_v = g_d[:].rearrange("(p n) k -> p n k", p=P)
    o_v = o_d[:].rearrange("(p n) -> p n", p=P)

    with TileContext(nc) as tc:
        import contextlib
        rep_loop = tc.For_i(0, reps, 1) if reps > 1 else contextlib.nullcontext()
        with (
            rep_loop,
            tc.tile_pool(name="io", bufs=2) as io,
            tc.tile_pool(name="etmp", bufs=1) as etmp,
            tc.tile_pool(name="constp", bufs=1) as constp,
            tc.tile_pool(name="work", bufs=1) as work,
            tc.tile_pool(name="qpool", bufs=1) as qpool,
            tc.tile_pool(name="accp", bufs=1) as accp,
            tc.tile_pool(name="outp", bufs=2) as outp,
        ):
            nln64 = constp.tile([P, 1], F32, tag="nln64")
            nc.gpsimd.memset(nln64, -LN64)
            for j in range(N_TILES):
                sl = slice(j * R, (j + 1) * R)

                a_t = io.tile([P, R, 8], F32, tag="a")
                b_t = io.tile([P, R, 8], F32, tag="b")
                g_t = io.tile([P, R, 8], F32, tag="g")
                nc.sync.dma_start(out=a_t, in_=a_v[:, sl, :])
                nc.sync.dma_start(out=b_t, in_=b_v[:, sl, :])
                nc.sync.dma_start(out=g_t, in_=g_v[:, sl, :])

                # --- stage 1: softmax denominators & logs ---
                # exp in fp16 scaled by 1/64 (bias -ln64; keeps sums and
                # Sa*Sb in fp16 range; the Ln `scale` args cancel it
                # exactly). Sums via an in-place fp16 add-tree over the
                # last dim (2x_1p) -- reduce_sum always runs 1x.
                sums = {}
                eg_t = None
                for name, x_t in (("g", g_t), ("a", a_t), ("b", b_t)):
                    e_t = etmp.tile([P, R, 8], F16, tag="e" + name,
                                    name=f"e{name}_{j}")
                    nc.scalar.activation(out=e_t, in_=x_t, func=AF.Exp,
                                         bias=nln64[:, :])
                    # tree into the high half: [:, :, 4:8] += [:, :, 0:4]
                    nc.vector.tensor_add(e_t[:, :, 4:8], e_t[:, :, 4:8],
                                         e_t[:, :, 0:4])
                    nc.vector.tensor_add(e_t[:, :, 6:8], e_t[:, :, 6:8],
                                         e_t[:, :, 4:6])
                    nc.vector.tensor_add(e_t[:, :, 7], e_t[:, :, 7],
                                         e_t[:, :, 6])
                    sums[name] = e_t[:, :, 7]
                    if name == "g":
                        eg_t = e_t
                sg_t = sums["g"]

                # S = lsg - lsa - lsb = ln(64*Sg') - ln(4096*Sa'Sb')
                sab = work.tile([P, R], F16, tag="sab")
                nc.vector.tensor_mul(sab, sums["a"], sums["b"])
                lsab = work.tile([P, R], F16, tag="lsab")
                nc.scalar.activation(out=lsab, in_=sab, func=AF.Ln,
                                     scale=4096.0)
                lsg = work.tile([P, R], F16, tag="lsg")
                nc.scalar.activation(out=lsg, in_=sg_t, func=AF.Ln,
                                     scale=64.0)
                s_t = work.tile([P, R], F16, tag="S")
                nc.vector.tensor_sub(s_t, lsg, lsab)

                # q' tiles: pos g: q' = gamma[g] - S ; not g: q' = ln(sum_g - e_g[g]) - S
                # Written in TERM_DT (fp16): halves DVE time of the term
                # stage via 2x_1P mode; error ~5e-4 rel (simulated).
                q = {}
                for gidx in _POS_GS:
                    qt = qpool.tile([P, R], TERM_DT, tag=f"qp{gidx}")
                    nc.vector.tensor_sub(qt, g_t[:, :, gidx], s_t)
                    q[(gidx, 0)] = qt
                for gidx in _NOT_GS:
                    wp = work.tile([P, R], F16, tag="wpre")
                    nc.vector.tensor_sub(wp, sg_t, eg_t[:, :, gidx])
                    wl = work.tile([P, R], F16, tag="wlog")
                    nc.scalar.activation(out=wl, in_=wp, func=AF.Ln,
                                         scale=64.0)
                    qt = qpool.tile([P, R], TERM_DT, tag=f"qn{gidx}")
                    nc.vector.tensor_sub(qt, wl, s_t)
                    q[(gidx, 1)] = qt

                # --- stage 2: 36 terms, all fp16 2x/4x DVE ---
                # P tile: 14 pair sums (fp32 in -> fp16 out, 1x).
                p16 = work.tile([P, 14, R], TERM_DT, tag="p16")
                for i, (ai, bi) in enumerate(_PAIR_SLOTS):
                    nc.vector.tensor_add(p16[:, i, :],
                                         a_t[:, :, ai], b_t[:, :, bi])

                # D tile: one batched subtract per q-group (fp16 TT, 2x).
                d36 = work.tile([P, 36, R], TERM_DT, tag="d36")
                for qkey, p0, p1, d0 in _QGROUPS:
                    n = p1 - p0
                    nc.vector.tensor_sub(
                        d36[:, d0:d0 + n, :], p16[:, p0:p1, :],
                        _bcast_mid(q[qkey], n),
                    )

                # relu in place, split across engines: DVE does the first
                # half (tensor_scalar 4x), ScalarE the second half (it is
                # otherwise idle while DVE is the bottleneck). Then pairwise
                # tree sum (fp16 TT adds, 2x), final level in fp32.
                # relu fully on ScalarE: it has slack (1 table load now) and
                # the d36 chain was measured non-binding; frees DVE cycles.
                nc.scalar.activation(out=d36, in_=d36, func=AF.Relu)
                # Fold DVE's third first (no wait on ACT), then ACT's share.
                nc.vector.tensor_add(d36[:, 0:6, :], d36[:, 0:6, :],
                                     d36[:, 6:12, :])
                nc.vector.tensor_add(d36[:, 12:24, :], d36[:, 12:24, :],
                                     d36[:, 24:36, :])
                nc.vector.tensor_add(d36[:, 12:18, :], d36[:, 12:18, :],
                                     d36[:, 18:24, :])
                nc.vector.tensor_add(d36[:, 0:6, :], d36[:, 0:6, :],
                                     d36[:, 12:18, :])
                nc.vector.tensor_add(d36[:, 0:3, :], d36[:, 0:3, :],
                                     d36[:, 3:6, :])
                nc.vector.tensor_add(d36[:, 0, :], d36[:, 0, :], d36[:, 1, :])
                loss_t = outp.tile([P, R], F32, tag="loss")
                nc.vector.tensor_add(loss_t, d36[:, 0, :], d36[:, 2, :])
                nc.sync.dma_start(out=o_v[:, sl], in_=loss_t)

    nc.compile()
    return nc


_NC_CACHE = None


def _get_nc():
    global _NC_CACHE
    if _NC_CACHE is None:
        _NC_CACHE = build_kernel()
    return _NC_CACHE


def kernel(alpha_logits, beta_logits, gamma_logits, _trace=False):
    nc = _get_nc()
    in_maps = []
    for c in range(N_CORES):
        sl = slice(c * B_CORE, (c + 1) * B_CORE)
        in_maps.append({
            "alpha_logits": np.ascontiguousarray(alpha_logits[sl]),
            "beta_logits": np.ascontiguousarray(beta_logits[sl]),
            "gamma_logits": np.ascontiguousarray(gamma_logits[sl]),
        })
    res = run_bass_kernel_spmd(nc, in_maps, core_ids=list(range(N_CORES)),
                               trace=_trace)
    out = np.concatenate([r["loss"] for r in res.results])
    if _trace:
        kernel.last_result = res
    return out



# revision 14
# speedup vs baseline: 1.3621x; 1.0391x over previous
"""Trainium2 Bass kernel for nn_CrossCategoryLoss.

loss(row) = sum_t relu(log_a[A_t] + log_b[B_t] - c_t)
  with c_t = log_g[G_t] (pos) or log(1 - exp(log_g[G_t])) (not).

Rewrites used (all per-row, exact in fp32 up to rounding):
  log_a[i] = alpha[i] - lsa,  lsa = ln(sum_j exp(alpha[j]))  (no max-sub
  needed: inputs are N(0,1), |x| < ~6, exp is safe in fp32)
  log(1-exp(log_g[k])) = ln(sum_g - exp(gamma[k])) - lsg
  term_t = relu(alpha[A] + beta[B] - q_t + S),  S = lsg - lsa - lsb
         = relu(p_AB - q'_t)   with p_AB = alpha[A]+beta[B], q'_t = q_t - S
    q_t = gamma[G]  (pos)  or  ln(sum_g - exp(gamma[G]))  (not)

Sharding: pure data-parallel over 8 cores; each core handles B/8 rows.
Per-core layout: rows viewed as [128 partitions, 4096 rows], tiles of
R rows per partition.
"""

import numpy as np

import concourse.bass as bass
import concourse.bacc as bacc
import concourse.mybir as mybir
from concourse.tile import TileContext
from concourse.bass_utils import run_bass_kernel_spmd

N_CORES = 8
B = 4194304
B_CORE = B // N_CORES          # 524288 rows per core
P = 128                        # partitions
ROWS_PER_PART = B_CORE // P    # 4096
R = 512                        # rows per partition per tile
N_TILES = ROWS_PER_PART // R   # 8

F32 = mybir.dt.float32
AX = mybir.AxisListType
AF = mybir.ActivationFunctionType
OP = mybir.AluOpType

# (alpha_idx, beta_idx, gamma_idx, is_not) - 36 constraint terms.
_TERMS = [
    (0, 4, 4, 0), (0, 4, 1, 1), (0, 4, 2, 1),
    (0, 6, 4, 0), (0, 6, 1, 1), (0, 6, 2, 1),
    (1, 5, 5, 0), (1, 5, 0, 1), (1, 5, 2, 1),
    (1, 6, 5, 0), (1, 6, 0, 1), (1, 6, 2, 1),
    (2, 4, 4, 0), (2, 4, 1, 1), (2, 4, 2, 1),
    (2, 5, 5, 0), (2, 5, 0, 1), (2, 5, 2, 1),
    (2, 6, 6, 0), (2, 7, 7, 0), (2, 7, 2, 1),
    (4, 0, 4, 0), (4, 0, 1, 1), (4, 0, 2, 1),
    (4, 2, 4, 0), (4, 2, 1, 1), (4, 2, 2, 1),
    (5, 1, 5, 0), (5, 1, 0, 1), (5, 1, 2, 1),
    (5, 2, 5, 0), (5, 2, 0, 1), (5, 2, 2, 1),
    (6, 2, 6, 0), (7, 2, 7, 0), (7, 2, 2, 1),
]

# Group terms by (a, b) pair, preserving first-appearance order.
_PAIRS: list[tuple[int, int]] = []
_PAIR_TERMS: dict[tuple[int, int], list[tuple[int, int]]] = {}
for _a, _b, _g, _n in _TERMS:
    if (_a, _b) not in _PAIR_TERMS:
        _PAIRS.append((_a, _b))
        _PAIR_TERMS[(_a, _b)] = []
    _PAIR_TERMS[(_a, _b)].append((_g, _n))

_NOT_GS = sorted({g for _, _, g, n in _TERMS if n})      # [0, 1, 2]
_POS_GS = sorted({g for _, _, g, n in _TERMS if not n})  # [4, 5, 6, 7]

# fp16 for the term stage: 16-bit dtype unlocks the DVE 2x_1P perf mode on
# tensor_tensor / tensor_scalar ops (fp32 is capped at 1x). Simulated error
# vs fp64 reference: norm-rel ~5e-4, absmax/scale ~6e-4. Stage 1 (exp sums,
# logs, S) stays fp32.
TERM_DT = mybir.dt.float16

# Pair slots in the P tile, ordered so every q-group's pair set is a
# contiguous slot range (enables one batched subtract per q-group):
_PAIR_SLOTS = [
    (0, 4), (0, 6), (2, 4), (4, 0), (4, 2),      # X1: q-triple {g4, w1, w2}
    (1, 5), (1, 6), (2, 5), (5, 1), (5, 2),      # X2: q-triple {g5, w0, w2}
    (2, 7), (7, 2), (2, 6), (6, 2),              # tail
]
# (q_key, pair_slot_range, d_slot_start): D[d0:d0+n] = P[p0:p1] - q
_QGROUPS = [
    ((4, 0), 0, 5, 0),     # g4:  D[0:5]   = P[0:5]  - q_g4
    ((1, 1), 0, 5, 5),     # w1:  D[5:10]  = P[0:5]  - q_w1
    ((2, 1), 0, 12, 10),   # w2:  D[10:22] = P[0:12] - q_w2
    ((5, 0), 5, 10, 22),   # g5:  D[22:27] = P[5:10] - q_g5
    ((0, 1), 5, 10, 27),   # w0:  D[27:32] = P[5:10] - q_w0
    ((7, 0), 10, 12, 32),  # g7:  D[32:34] = P[10:12]- q_g7
    ((6, 0), 12, 14, 34),  # g6:  D[34:36] = P[12:14]- q_g6
]


def _bcast_mid(ap, n):
    """[P, R] access pattern -> [P, n, R] with a zero-stride middle dim."""
    a = ap[:, :]
    return bass.AP(tensor=a.tensor, offset=a.offset,
                   ap=[a.ap[0], [0, n], a.ap[1]])


def _swap_free(ap):
    """View a [P, K, R] tile iterated as [P, R, K] (same memory)."""
    a = ap[:, :, :]
    return bass.AP(tensor=a.tensor, offset=a.offset,
                   ap=[a.ap[0], a.ap[2], a.ap[1]])


def build_kernel(reps: int = 1) -> bass.Bass:
    nc = bacc.Bacc("TRN2", target_bir_lowering=False, debug=False,
                   num_devices=N_CORES)

    # The default table chooser alternates ACT function-table sets between
    # Exp and Ln ops (measured: 18 LoadActFuncSet = ~45us of ~2.7us reloads
    # per kernel). All functions used here (Exp, Ln, Relu) coexist in
    # "natural_log_exp_and_others", so restrict the chooser to that set.
    # Other entries are blanked (not removed) to keep act_func_set_id
    # indices aligned with act_info.json.
    _orig_tables = bacc.get_activation_tables

    def _one_set(arch):
        return {
            name: (fns if name == "natural_log_exp_and_others" else set())
            for name, fns in _orig_tables(arch).items()
        }

    bacc.get_activation_tables = _one_set
    try:
        return _build_body(nc, reps)
    finally:
        bacc.get_activation_tables = _orig_tables


def _build_body(nc, reps: int) -> bass.Bass:

    a_d = nc.dram_tensor("alpha_logits", [B_CORE, 8], F32, kind="ExternalInput")
    b_d = nc.dram_tensor("beta_logits", [B_CORE, 8], F32, kind="ExternalInput")
    g_d = nc.dram_tensor("gamma_logits", [B_CORE, 8], F32, kind="ExternalInput")
    o_d = nc.dram_tensor("loss", [B_CORE], F32, kind="ExternalOutput")

    a_v = a_d[:].rearrange("(p n) k -> p n k", p=P)
    b_v = b_d[:].rearrange("(p n) k -> p n k", p=P)
    g_v = g_d[:].rearrange("(p n) k -> p n k", p=P)
    o_v = o_d[:].rearrange("(p n) -> p n", p=P)

    with TileContext(nc) as tc:
        import contextlib
        rep_loop = tc.For_i(0, reps, 1) if reps > 1 else contextlib.nullcontext()
        with (
            rep_loop,
            tc.tile_pool(name="io", bufs=2) as io,
            tc.tile_pool(name="etmp", bufs=1) as etmp,
            tc.tile_pool(name="epsum", bufs=1, space="PSUM") as epsum,
            tc.tile_pool(name="work", bufs=1) as work,
            tc.tile_pool(name="qpool", bufs=1) as qpool,
            tc.tile_pool(name="accp", bufs=1) as accp,
            tc.tile_pool(name="outp", bufs=2) as outp,
        ):
            for j in range(N_TILES):
                sl = slice(j * R, (j + 1) * R)

                a_t = io.tile([P, R, 8], F32, tag="a")
                b_t = io.tile([P, R, 8], F32, tag="b")
                g_t = io.tile([P, R, 8], F32, tag="g")
                nc.sync.dma_start(out=a_t, in_=a_v[:, sl, :])
                nc.sync.dma_start(out=b_t, in_=b_v[:, sl, :])
                nc.sync.dma_start(out=g_t, in_=g_v[:, sl, :])

                # --- stage 1: softmax denominators & logs (fp32) ---
                # exp scratch rotates through PSUM (a, g) and SBUF (b):
                # PSUM is otherwise unused here (no matmuls), ScalarE writes
                # it faster than SBUF, and the two spaces let exp_b proceed
                # while reduce_a still reads the PSUM slot.
                sums32 = {}
                eg_t = None
                for name, x_t in (("a", a_t), ("b", b_t), ("g", g_t)):
                    pool = etmp if name == "b" else epsum
                    e_t = pool.tile([P, R, 8], F32, tag="e", name=f"e{name}_{j}")
                    nc.scalar.activation(out=e_t, in_=x_t, func=AF.Exp)
                    s_t = work.tile([P, R], F32, tag="s" + name)
                    nc.vector.reduce_sum(out=s_t, in_=e_t, axis=AX.X)
                    sums32[name] = s_t
                    if name == "g":
                        eg_t = e_t
                sg_t = sums32["g"]

                # S = lsg - lsa - lsb = ln(sum_g) - ln(sum_a * sum_b)
                sab = work.tile([P, R], F32, tag="sab")
                nc.vector.tensor_mul(sab, sums32["a"], sums32["b"])
                lsab = work.tile([P, R], F32, tag="lsab")
                nc.scalar.activation(out=lsab, in_=sab, func=AF.Ln)
                lsg = work.tile([P, R], F32, tag="lsg")
                nc.scalar.activation(out=lsg, in_=sg_t, func=AF.Ln)
                s_t = work.tile([P, R], F32, tag="S")
                nc.vector.tensor_sub(s_t, lsg, lsab)

                # q' tiles: pos g: q' = gamma[g] - S ; not g: q' = ln(sum_g - e_g[g]) - S
                # Written in TERM_DT (fp16): halves DVE time of the term
                # stage via 2x_1P mode; error ~5e-4 rel (simulated).
                q = {}
                for gidx in _POS_GS:
                    qt = qpool.tile([P, R], TERM_DT, tag=f"qp{gidx}")
                    nc.vector.tensor_sub(qt, g_t[:, :, gidx], s_t)
                    q[(gidx, 0)] = qt
                for gidx in _NOT_GS:
                    wp = work.tile([P, R], F32, tag="wpre")
                    nc.vector.tensor_sub(wp, sg_t, eg_t[:, :, gidx])
                    wl = work.tile([P, R], F32, tag="wlog")
                    nc.scalar.activation(out=wl, in_=wp, func=AF.Ln)
                    qt = qpool.tile([P, R], TERM_DT, tag=f"qn{gidx}")
                    nc.vector.tensor_sub(qt, wl, s_t)
                    q[(gidx, 1)] = qt

                # --- stage 2: 36 terms, all fp16 2x/4x DVE ---
                # P tile: 14 pair sums (fp32 in -> fp16 out, 1x).
                p16 = work.tile([P, 14, R], TERM_DT, tag="p16")
                for i, (ai, bi) in enumerate(_PAIR_SLOTS):
                    nc.vector.tensor_add(p16[:, i, :],
                                         a_t[:, :, ai], b_t[:, :, bi])

                # D tile: one batched subtract per q-group (fp16 TT, 2x).
                d36 = work.tile([P, 36, R], TERM_DT, tag="d36")
                for qkey, p0, p1, d0 in _QGROUPS:
                    n = p1 - p0
                    nc.vector.tensor_sub(
                        d36[:, d0:d0 + n, :], p16[:, p0:p1, :],
                        _bcast_mid(q[qkey], n),
                    )

                # relu in place, split across engines: DVE does the first
                # half (tensor_scalar 4x), ScalarE the second half (it is
                # otherwise idle while DVE is the bottleneck). Then pairwise
                # tree sum (fp16 TT adds, 2x), final level in fp32.
                # relu fully on ScalarE: it has slack (1 table load now) and
                # the d36 chain was measured non-binding; frees DVE cycles.
                nc.scalar.activation(out=d36, in_=d36, func=AF.Relu)
                # Fold DVE's third first (no wait on ACT), then ACT's share.
                nc.vector.tensor_add(d36[:, 0:6, :], d36[:, 0:6, :],
                                     d36[:, 6:12, :])
                nc.vector.tensor_add(d36[:, 12:24, :], d36[:, 12:24, :],
                                     d36[:, 24:36, :])
                nc.vector.tensor_add(d36[:, 12:18, :], d36[:, 12:18, :],
                                     d36[:, 18:24, :])
                nc.vector.tensor_add(d36[:, 0:6, :], d36[:, 0:6, :],
                                     d36[:, 12:18, :])
                nc.vector.tensor_add(d36[:, 0:3, :], d36[:, 0:3, :],
                                     d36[:, 3:6, :])
                nc.vector.tensor_add(d36[:, 0, :], d36[:, 0, :], d36[:, 1, :])
                loss_t = outp.tile([P, R], F32, tag="loss")
                nc.vector.tensor_add(loss_t, d36[:, 0, :], d36[:, 2, :])
                nc.sync.dma_start(out=o_v[:, sl], in_=loss_t)

    nc.compile()
    return nc


_NC_CACHE = None


def _get_nc():
    global _NC_CACHE
    if _NC_CACHE is None:
        _NC_CACHE = build_kernel()
    return _NC_CACHE


def kernel(alpha_logits, beta_logits, gamma_logits, _trace=False):
    nc = _get_nc()
    in_maps = []
    for c in range(N_CORES):
        sl = slice(c * B_CORE, (c + 1) * B_CORE)
        in_maps.append({
            "alpha_logits": np.ascontiguousarray(alpha_logits[sl]),
            "beta_logits": np.ascontiguousarray(beta_logits[sl]),
            "gamma_logits": np.ascontiguousarray(gamma_logits[sl]),
        })
    res = run_bass_kernel_spmd(nc, in_maps, core_ids=list(range(N_CORES)),
                               trace=_trace)
    out = np.concatenate([r["loss"] for r in res.results])
    if _trace:
        kernel.last_result = res
    return out

